# revision 1
# baseline (speedup 1.0000x reference)
"""Trainium2 Bass kernel for nn_AttentionFFM.

Reference computation, per token (b, k) with v = x[b, :, k] (a 64-vector)
and constant w = vk @ vk.T (64x64, symmetric):

    s_ij   = v_i * v_j
    z_ij   = s_ij * w_ij
    out_i  = (sum_j exp(z_ij) * s_ij) / (sum_j exp(z_ij))

(the v_i / v_j softmax-weighting factors are absorbed exactly by using
s inside the numerator sum; softmax max-subtraction is skipped since
|z| < ~11 for these inputs, well within fp32/bf16 exp range).

Layout (per core; batch-parallel across 8 cores, 128 batches each):
  - partitions = batch b (128), free = (i, j) for one k-slice at a time
  - the x tile [128, 1024] is x[b] contiguous; v_i and v_j enter the
    64x64 outer-product via stride tricks (i: step 16 / broadcast 0,
    j: broadcast 0 / step 16) on that one tile -- no transposes.
  - s, z, e=exp(z), q=e*s are bf16 [128, 4096]; row-sums of e and q are
    pairwise-halving trees of dense 2x-mode bf16 adds; final level,
    reciprocal and the output multiply are fp32.
  - All working tiles are allocated ONCE and ping-ponged by k parity.
    (Tile-pool slot reuse triggers a hardware fault/hang in this
    environment, so no per-iteration pool.tile() allocations.)
"""

import sys
from contextlib import ExitStack

import numpy as np

if "/opt/trn_rl_repo" not in sys.path:
    sys.path.insert(0, "/opt/trn_rl_repo")

import concourse.bass as bass
import concourse.tile as tile
from concourse import bacc, mybir
from concourse.bass_utils import run_bass_kernel_spmd

# Optional NEFF compile cache (keyed by BIR hash, traceback metadata
# stripped) — skips the multi-minute walrus compile when this exact kernel
# was compiled before on this machine. Falls back to a normal compile.
_NEFF_CACHE_DIR = "/tmp/bass_neff_cache"


def _install_neff_cache():
    import hashlib
    import shutil

    from concourse import bass_utils as _bu

    if getattr(_bu.compile_bir_kernel, "_is_cached_wrapper", False):
        return

    _orig = _bu.compile_bir_kernel

    _volatile = {"ant_traceback", "filename", "lineno", "kernel_name"}

    def _strip(obj):
        if isinstance(obj, dict):
            return {k: _strip(v) for k, v in obj.items() if k not in _volatile}
        if isinstance(obj, list):
            return [_strip(v) for v in obj]
        return obj

    def _key(bir_json):
        import orjson

        try:
            normalized = orjson.dumps(_strip(orjson.loads(bir_json)))
        except Exception:
            normalized = bir_json
        return hashlib.sha256(normalized).hexdigest()[:32]

    def _cached(bir_json, tmpdir, neff_name="file.neff"):
        import os as _os

        try:
            _os.makedirs(_NEFF_CACHE_DIR, exist_ok=True)
            p = _os.path.join(_NEFF_CACHE_DIR, _key(bir_json) + ".neff")
            dst = _os.path.join(tmpdir, neff_name)
            if _os.path.exists(p):
                shutil.copy(p, dst)
                return dst
            out = _orig(bir_json, tmpdir, neff_name)
            try:
                shutil.copy(out, p)
            except Exception:
                pass
            return out
        except Exception:
            return _orig(bir_json, tmpdir, neff_name)

    _cached._is_cached_wrapper = True
    _bu.compile_bir_kernel = _cached
    try:
        import concourse.bass2jax as _b2j

        if hasattr(_b2j, "compile_bir_kernel"):
            _b2j.compile_bir_kernel = _cached
    except Exception:
        pass


_install_neff_cache()

B, M, K = 1024, 64, 16
NCORES = 8
BL = B // NCORES  # batches per core

_CACHE = {}
LAST_RESULTS = None
TRACE = False

# Debug/bisect knobs (only for local testing; defaults = production kernel).
K_LIMIT = K
LINEARIZE = False
NBUF = 2  # parity buffers for working tiles
S_ENGINE = "vector"  # "vector" | "gpsimd" — engine for the s outer-product
X_COPY = False  # read v_j from a duplicate x tile (avoid same-tensor 2-port read)


def _tree_tiles(pool, prefix):
    """Pre-allocate the pairwise-reduction level tiles for one tensor."""
    tiles = {}
    width = M // 2
    while width >= 2:
        tiles[width] = pool.tile(
            [BL, M, width], mybir.dt.bfloat16, tag=f"{prefix}{width}",
            name=f"{prefix}{width}",
        )
        width //= 2
    tiles["res"] = pool.tile(
        [BL, M], mybir.dt.float32, tag=f"{prefix}r", name=f"{prefix}r"
    )
    return tiles


def _reduce_tree(nc, t, tiles):
    """Row-sums over j of t [BL, M, M] (bf16) -> tiles['res'] [BL, M] fp32."""
    cur = t
    width = M // 2
    while width >= 2:
        nxt = tiles[width]
        nc.vector.tensor_tensor(
            out=nxt[:, :, :],
            in0=cur[:, :, 0:width],
            in1=cur[:, :, width : 2 * width],
            op=mybir.AluOpType.add,
        )
        cur = nxt
        width //= 2
    res = tiles["res"]
    nc.vector.tensor_tensor(
        out=res[:, :],
        in0=cur[:, :, 0],
        in1=cur[:, :, 1],
        op=mybir.AluOpType.add,
    )
    return res


def _build():
    nc = bacc.Bacc(
        "TRN2",
        target_bir_lowering=False,
        debug=False,
        num_devices=NCORES,
    )
    x_in = nc.declare_dram_parameter("x", [BL, M * K], mybir.dt.float32, isOutput=False)
    w_in = nc.declare_dram_parameter(
        "w", [1, M * M], mybir.dt.bfloat16, isOutput=False
    )
    out_ext = nc.declare_dram_parameter(
        "out", [BL, M * K], mybir.dt.float32, isOutput=True
    )

    with tile.TileContext(nc, linearize=LINEARIZE) as tc, ExitStack() as ctx:
        const = ctx.enter_context(tc.tile_pool(name="const", bufs=1))
        big = ctx.enter_context(tc.tile_pool(name="big", bufs=1))
        trees = ctx.enter_context(tc.tile_pool(name="trees", bufs=1))

        x_sb = const.tile([BL, M * K], mybir.dt.float32)
        nc.sync.dma_start(out=x_sb[:, :], in_=x_in[:, :])
        if X_COPY:
            x_sb2 = const.tile([BL, M * K], mybir.dt.float32)
            nc.sync.dma_start(out=x_sb2[:, :], in_=x_in[:, :])
        else:
            x_sb2 = x_sb

        w_bf = const.tile([BL, M * M], mybir.dt.bfloat16)
        w_bcast = bass.AP(
            tensor=w_in[0:1, :].tensor,
            offset=w_in[0:1, :].offset,
            ap=[[0, BL], [1, M * M]],
        )
        nc.gpsimd.dma_start(out=w_bf[:, :], in_=w_bcast)

        out_sb = const.tile([BL, M * K], mybir.dt.float32)
        out_3d = out_sb[:, :].rearrange("p (i k) -> p i k", k=K)
        x_3d = x_sb[:, :].rearrange("p (i k) -> p i k", k=K)
        x2_3d = x_sb2[:, :].rearrange("p (i k) -> p i k", k=K)

        # Pre-allocated ping-pong working tiles (no pool slot cycling).
        s_t = [
            big.tile([BL, M, M], mybir.dt.bfloat16, tag=f"s{p}", name=f"s{p}")
            for p in range(NBUF)
        ]
        z_t = [
            big.tile([BL, M * M], mybir.dt.bfloat16, tag=f"z{p}", name=f"z{p}")
            for p in range(NBUF)
        ]
        e_t = [
            big.tile([BL, M, M], mybir.dt.bfloat16, tag=f"e{p}", name=f"e{p}")
            for p in range(NBUF)
        ]
        q_t = [
            big.tile([BL, M, M], mybir.dt.bfloat16, tag=f"q{p}", name=f"q{p}")
            for p in range(NBUF)
        ]
        dt_t = [_tree_tiles(trees, f"d{p}") for p in range(NBUF)]
        nt_t = [_tree_tiles(trees, f"n{p}") for p in range(NBUF)]
        rd_t = [
            trees.tile([BL, M], mybir.dt.float32, tag=f"rd{p}", name=f"rd{p}")
            for p in range(NBUF)
        ]

        for k in range(K_LIMIT):
            p = k % NBUF
            xk = x_3d[:, :, k]  # [BL, M] view of v (strided by K)
            xi = xk.unsqueeze(-1).broadcast_to((BL, M, M))
            xj = x2_3d[:, :, k].unsqueeze(1).broadcast_to((BL, M, M))

            s = s_t[p]
            s_eng = nc.gpsimd if S_ENGINE == "gpsimd" else nc.vector
            s_eng.tensor_tensor(
                out=s[:, :, :], in0=xi, in1=xj, op=mybir.AluOpType.mult
            )
            s_flat = s[:, :, :].rearrange("p i j -> p (i j)")

            z = z_t[p]
            nc.vector.tensor_tensor(
                out=z[:, :], in0=s_flat, in1=w_bf[:, :], op=mybir.AluOpType.mult
            )

            e = e_t[p]
            nc.scalar.activation(
                out=e[:, :, :].rearrange("p i j -> p (i j)"),
                in_=z[:, :],
                func=mybir.ActivationFunctionType.Exp,
            )

            q = q_t[p]
            nc.vector.tensor_tensor(
                out=q[:, :, :].rearrange("p i j -> p (i j)"),
                in0=e[:, :, :].rearrange("p i j -> p (i j)"),
                in1=s_flat,
                op=mybir.AluOpType.mult,
            )

            denom = _reduce_tree(nc, e, dt_t[p])
            numer = _reduce_tree(nc, q, nt_t[p])

            rdenom = rd_t[p]
            nc.vector.reciprocal(out=rdenom[:, :], in_=denom[:, :])
            nc.vector.tensor_tensor(
                out=out_3d[:, :, k],
                in0=numer[:, :],
                in1=rdenom[:, :],
                op=mybir.AluOpType.mult,
            )

        nc.sync.dma_start(out=out_ext[:, :], in_=out_sb[:, :])

    nc.compile()
    return nc


def _get_nc():
    if "nc" not in _CACHE:
        _CACHE["nc"] = _build()
    return _CACHE["nc"]


def kernel(x, vk):
    global LAST_RESULTS
    x = np.ascontiguousarray(np.asarray(x), dtype=np.float32)
    vk = np.ascontiguousarray(np.asarray(vk), dtype=np.float32)
    assert x.shape == (B, M, K) and vk.shape[0] == M

    import ml_dtypes

    w = (vk @ vk.T).astype(ml_dtypes.bfloat16).reshape(1, M * M)
    xs = x.reshape(NCORES, BL, M * K)
    in_maps = [{"x": xs[i], "w": w} for i in range(NCORES)]

    nc = _get_nc()
    res = run_bass_kernel_spmd(nc, in_maps, core_ids=list(range(NCORES)), trace=TRACE)
    LAST_RESULTS = res
    out = np.concatenate(
        [np.asarray(res.results[i]["out"]).reshape(BL, M, K) for i in range(NCORES)],
        axis=0,
    )
    return out.astype(np.float32, copy=False)



# revision 13
# speedup vs baseline: 1.1342x; 1.1342x over previous
"""Trainium2 Bass kernel for nn_AttentionFFM — restructured v1.

Reference, per token (b, k) with v = x[b, :, k] (64-vector) and
constant symmetric w = vk @ vk.T:

    e_ij  = exp(v_i * v_j * w_ij)
    out_i = v_i * (sum_j v_j e_ij) / (sum_j e_ij)

(the v_i factor is pulled OUT of the softmax-weighted sum, so the
full-size q = s*e pass and the s = v_i*v_j outer product of the old
kernel disappear; only j-varying factors touch 64x64-sized tensors).

Layout (per core; batch-parallel across 8 cores, BL=128 batches each):
  partitions = batch b, free = (k2, i, j) for one k-PAIR at a time
  (k-pair packing halves instruction count). All big ops are bf16 with
  stride-1 innermost APs => DVE 2x mode:
    m1 = w  (*) v_j-broadcast        [DVE, fast AP]
    z  = m1 (*) vi_rep               [DVE, vi_rep materialized on GpSimd]
    e  = exp(z)                      [ACT]
    t  = e  (*) v_j-broadcast        [DVE, overwrites m1]
    D  = reduce_j(e), N = reduce_j(t)  [DVE native tensor_reduce, bf16 out]
  Final (batched over all 16 k): out = v * N * reciprocal(D) in fp32.

Host-side prep: x is passed k-major ((k,i) columns, bf16) so v-vectors
are contiguous per k; output is returned k-major and transposed back on
host. All tiles are allocated ONCE (tile-pool slot cycling faulted on
this HW in a previous session).
"""

import sys
from contextlib import ExitStack

import numpy as np

if "/opt/trn_rl_repo" not in sys.path:
    sys.path.insert(0, "/opt/trn_rl_repo")

import concourse.bass as bass
import concourse.tile as tile
from concourse import bacc, mybir
from concourse.bass_utils import run_bass_kernel_spmd

_NEFF_CACHE_DIR = "/tmp/bass_neff_cache"


def _install_neff_cache():
    import hashlib
    import shutil

    from concourse import bass_utils as _bu

    if getattr(_bu.compile_bir_kernel, "_is_cached_wrapper", False):
        return

    _orig = _bu.compile_bir_kernel

    _volatile = {"ant_traceback", "filename", "lineno", "kernel_name"}

    def _strip(obj):
        if isinstance(obj, dict):
            return {k: _strip(v) for k, v in obj.items() if k not in _volatile}
        if isinstance(obj, list):
            return [_strip(v) for v in obj]
        return obj

    def _key(bir_json):
        import orjson

        try:
            normalized = orjson.dumps(_strip(orjson.loads(bir_json)))
        except Exception:
            normalized = bir_json
        return hashlib.sha256(normalized).hexdigest()[:32]

    def _cached(bir_json, tmpdir, neff_name="file.neff"):
        import os as _os

        try:
            _os.makedirs(_NEFF_CACHE_DIR, exist_ok=True)
            p = _os.path.join(_NEFF_CACHE_DIR, _key(bir_json) + ".neff")
            dst = _os.path.join(tmpdir, neff_name)
            if _os.path.exists(p):
                shutil.copy(p, dst)
                return dst
            out = _orig(bir_json, tmpdir, neff_name)
            try:
                shutil.copy(out, p)
            except Exception:
                pass
            return out
        except Exception:
            return _orig(bir_json, tmpdir, neff_name)

    _cached._is_cached_wrapper = True
    _bu.compile_bir_kernel = _cached
    try:
        import concourse.bass2jax as _b2j

        if hasattr(_b2j, "compile_bir_kernel"):
            _b2j.compile_bir_kernel = _cached
    except Exception:
        pass


_install_neff_cache()

B, M, K = 1024, 64, 16
NCORES = 8
BL = B // NCORES
NPAIR = K // 2

_CACHE = {}
LAST_RESULTS = None
TRACE = False
PROBES = False  # embed one-shot timing probe instructions
VI_ENGINE = "vector"  # "vector" (broadcast-AP, 1x) | "gpsimd" (materialize on Pool)
REDUCE = "native"  # "native" (tensor_reduce) | "tree" (pairwise adds)


def _build():
    nc = bacc.Bacc(
        "TRN2",
        target_bir_lowering=False,
        debug=False,
        num_devices=NCORES,
    )
    # x, k-major bf16: xb[b, k*M + i] = x[b, i, k]
    x_in = nc.declare_dram_parameter(
        "x", [BL, K * M], mybir.dt.bfloat16, isOutput=False
    )
    w_in = nc.declare_dram_parameter(
        "w", [1, M * M], mybir.dt.bfloat16, isOutput=False
    )
    # out, k-major fp32: out[b, k*M + i]
    out_ext = nc.declare_dram_parameter(
        "out", [BL, K * M], mybir.dt.float32, isOutput=True
    )

    with tile.TileContext(nc) as tc, ExitStack() as ctx:
        const = ctx.enter_context(tc.tile_pool(name="const", bufs=1))
        big = ctx.enter_context(tc.tile_pool(name="big", bufs=1))

        xb_sb = const.tile([BL, K * M], mybir.dt.bfloat16)
        nc.sync.dma_start(out=xb_sb[:, :], in_=x_in[:, :])

        w_sb = const.tile([BL, M * M], mybir.dt.bfloat16)
        w_bcast = bass.AP(
            tensor=w_in[0:1, :].tensor,
            offset=w_in[0:1, :].offset,
            ap=[[0, BL], [1, M * M]],
        )
        nc.gpsimd.dma_start(out=w_sb[:, :], in_=w_bcast)

        out_sb = const.tile([BL, K * M], mybir.dt.float32)

        xb_3d = xb_sb[:, :].rearrange("p (k i) -> p k i", i=M)
        w4 = (
            w_sb[:, :]
            .rearrange("p (i j) -> p i j", j=M)
            .unsqueeze(1)
            .broadcast_to((BL, 2, M, M))
        )

        # Pre-allocated ping-pong working tiles.
        vi_t = [
            big.tile([BL, 2, M, M], mybir.dt.bfloat16, tag=f"vi{p}", name=f"vi{p}")
            for p in range(2)
        ] if VI_ENGINE == "gpsimd" else None
        mt_t = [
            big.tile([BL, 2, M, M], mybir.dt.bfloat16, tag=f"mt{p}", name=f"mt{p}")
            for p in range(2)
        ]
        z_t = [
            big.tile([BL, 2, M, M], mybir.dt.bfloat16, tag=f"z{p}", name=f"z{p}")
            for p in range(2)
        ]
        e_t = [
            big.tile([BL, 2, M, M], mybir.dt.bfloat16, tag=f"e{p}", name=f"e{p}")
            for p in range(2)
        ]
        tr_t = None
        if REDUCE == "tree":
            tr_t = []
            for p in range(2):
                pair = []
                for s in range(2):
                    tiles = {}
                    width = M // 2
                    while width >= 2:
                        tiles[width] = big.tile(
                            [BL, 2 * M, width],
                            mybir.dt.bfloat16,
                            tag=f"tr{p}{s}{width}",
                            name=f"tr{p}{s}{width}",
                        )
                        width //= 2
                    pair.append(tiles)
                tr_t.append(pair)

        d_all = const.tile([BL, K * M], mybir.dt.bfloat16, tag="dall", name="dall")
        n_all = const.tile([BL, K * M], mybir.dt.bfloat16, tag="nall", name="nall")
        rd_all = const.tile([BL, K * M], mybir.dt.float32, tag="rdall", name="rdall")
        p_all = const.tile([BL, K * M], mybir.dt.float32, tag="pall", name="pall")

        if PROBES:
            pr1 = const.tile([BL, 96], mybir.dt.float32, tag="pr1", name="pr1")
            pr4 = const.tile([BL, 3328], mybir.dt.bfloat16, tag="pr4", name="pr4")

        for g in range(NPAIR):
            par = g % 2
            xp = xb_3d[:, 2 * g : 2 * g + 2, :]  # [BL, 2, M] contiguous
            vj = xp.unsqueeze(-2).broadcast_to((BL, 2, M, M))  # j innermost, s=1
            vi = xp.unsqueeze(-1).broadcast_to((BL, 2, M, M))  # j innermost, s=0

            m1 = mt_t[par]
            nc.vector.tensor_tensor(
                out=m1[:, :, :, :], in0=w4, in1=vj, op=mybir.AluOpType.mult
            )

            z = z_t[par]
            if VI_ENGINE == "gpsimd":
                vi_rep = vi_t[par]
                nc.gpsimd.tensor_scalar(
                    out=vi_rep[:, :, :, :],
                    in0=vi,
                    scalar1=1.0,
                    scalar2=None,
                    op0=mybir.AluOpType.mult,
                    op1=mybir.AluOpType.bypass,
                )
                vi_in = vi_rep[:, :, :, :]
            else:
                vi_in = vi  # direct broadcast AP (1x mode, proven pattern)
            nc.vector.tensor_tensor(
                out=z[:, :, :, :],
                in0=m1[:, :, :, :],
                in1=vi_in,
                op=mybir.AluOpType.mult,
            )

            e = e_t[par]
            nc.scalar.activation(
                out=e[:, :, :, :].rearrange("p a i j -> p (a i j)"),
                in_=z[:, :, :, :].rearrange("p a i j -> p (a i j)"),
                func=mybir.ActivationFunctionType.Exp,
            )

            # t overwrites m1 (m1 is dead after z).
            t = mt_t[par]
            nc.vector.tensor_tensor(
                out=t[:, :, :, :],
                in0=e[:, :, :, :],
                in1=vj,
                op=mybir.AluOpType.mult,
            )

            e_r = e[:, :, :, :].rearrange("p a i j -> p (a i) j")
            t_r = t[:, :, :, :].rearrange("p a i j -> p (a i) j")
            cols = slice(g * 2 * M, (g + 1) * 2 * M)
            if REDUCE == "native":
                with nc.allow_low_precision("bf16 softmax sums, 64-term rows"):
                    nc.vector.tensor_reduce(
                        out=d_all[:, cols],
                        in_=e_r,
                        axis=mybir.AxisListType.X,
                        op=mybir.AluOpType.add,
                    )
                    nc.vector.tensor_reduce(
                        out=n_all[:, cols],
                        in_=t_r,
                        axis=mybir.AxisListType.X,
                        op=mybir.AluOpType.add,
                    )
            else:
                for src, dst, tt in ((e_r, d_all, tr_t[par][0]), (t_r, n_all, tr_t[par][1])):
                    cur = src
                    width = M // 2
                    while width >= 2:
                        nxt = tt[width][:, :, :]
                        nc.vector.tensor_tensor(
                            out=nxt,
                            in0=cur[:, :, 0:width],
                            in1=cur[:, :, width : 2 * width],
                            op=mybir.AluOpType.add,
                        )
                        cur = nxt
                        width //= 2
                    nc.vector.tensor_tensor(
                        out=dst[:, cols],
                        in0=cur[:, :, 0],
                        in1=cur[:, :, 1],
                        op=mybir.AluOpType.add,
                    )

            if PROBES and g == 0:
                # One-shot instruction-cost probes (distinct free sizes;
                # standard ISA ops only).
                w_flat = w_sb[:, :]
                nc.vector.tensor_reduce(
                    out=pr1[:, :],
                    in_=w_flat[:, 0:4032].rearrange("p (a b) -> p a b", b=42),
                    axis=mybir.AxisListType.X,
                    op=mybir.AluOpType.add,
                )
                nc.vector.tensor_scalar(
                    out=pr4[:, :],
                    in0=w_flat[:, 0:3328],
                    scalar1=1.00390625,
                    scalar2=None,
                    op0=mybir.AluOpType.mult,
                )

        nc.vector.reciprocal(out=rd_all[:, :], in_=d_all[:, :])
        nc.vector.tensor_tensor(
            out=p_all[:, :],
            in0=n_all[:, :],
            in1=rd_all[:, :],
            op=mybir.AluOpType.mult,
        )
        nc.vector.tensor_tensor(
            out=out_sb[:, :],
            in0=p_all[:, :],
            in1=xb_sb[:, :],
            op=mybir.AluOpType.mult,
        )

        nc.sync.dma_start(out=out_ext[:, :], in_=out_sb[:, :])

    nc.compile()
    return nc


def _get_nc():
    key = ("nc", PROBES, VI_ENGINE, REDUCE)
    if key not in _CACHE:
        _CACHE[key] = _build()
    return _CACHE[key]


def kernel(x, vk):
    global LAST_RESULTS
    x = np.ascontiguousarray(np.asarray(x), dtype=np.float32)
    vk = np.ascontiguousarray(np.asarray(vk), dtype=np.float32)
    assert x.shape == (B, M, K) and vk.shape[0] == M

    import ml_dtypes

    bf16 = ml_dtypes.bfloat16
    w = (vk @ vk.T).astype(bf16).reshape(1, M * M)
    # k-major bf16 per core: xb[b, k*M + i] = x[b, i, k]
    xs = x.reshape(NCORES, BL, M, K)
    in_maps = []
    for i in range(NCORES):
        xb = np.ascontiguousarray(xs[i].transpose(0, 2, 1)).reshape(BL, K * M)
        in_maps.append({"x": xb.astype(bf16), "w": w})

    nc = _get_nc()
    res = run_bass_kernel_spmd(nc, in_maps, core_ids=list(range(NCORES)), trace=TRACE)
    LAST_RESULTS = res
    outs = []
    for i in range(NCORES):
        o = np.asarray(res.results[i]["out"]).reshape(BL, K, M)
        outs.append(o.transpose(0, 2, 1))  # -> [BL, M, K]
    out = np.concatenate(outs, axis=0)
    return np.ascontiguousarray(out).astype(np.float32, copy=False)


# revision 20
# speedup vs baseline: 1.2811x; 1.1295x over previous
"""Trainium2 Bass kernel for nn_AttentionFFM — restructured v1.

Reference, per token (b, k) with v = x[b, :, k] (64-vector) and
constant symmetric w = vk @ vk.T:

    e_ij  = exp(v_i * v_j * w_ij)
    out_i = v_i * (sum_j v_j e_ij) / (sum_j e_ij)

(the v_i factor is pulled OUT of the softmax-weighted sum, so the
full-size q = s*e pass and the s = v_i*v_j outer product of the old
kernel disappear; only j-varying factors touch 64x64-sized tensors).

Layout (per core; batch-parallel across 8 cores, BL=128 batches each):
  partitions = batch b, free = (k2, i, j) for one k-PAIR at a time
  (k-pair packing halves instruction count). All big ops are bf16 with
  stride-1 innermost APs => DVE 2x mode:
    m1 = w  (*) v_j-broadcast        [DVE, fast AP]
    z  = m1 (*) vi_rep               [DVE, vi_rep materialized on GpSimd]
    e  = exp(z)                      [ACT]
    t  = e  (*) v_j-broadcast        [DVE, overwrites m1]
    D  = reduce_j(e), N = reduce_j(t)  [DVE native tensor_reduce, bf16 out]
  Final (batched over all 16 k): out = v * N * reciprocal(D) in fp32.

Host-side prep: x is passed k-major ((k,i) columns, bf16) so v-vectors
are contiguous per k; output is returned k-major and transposed back on
host. All tiles are allocated ONCE (tile-pool slot cycling faulted on
this HW in a previous session).
"""

import sys
from contextlib import ExitStack

import numpy as np

if "/opt/trn_rl_repo" not in sys.path:
    sys.path.insert(0, "/opt/trn_rl_repo")

import concourse.bass as bass
import concourse.tile as tile
from concourse import bacc, mybir
from concourse.bass_utils import run_bass_kernel_spmd

_NEFF_CACHE_DIR = "/tmp/bass_neff_cache"


def _install_neff_cache():
    import hashlib
    import shutil

    from concourse import bass_utils as _bu

    if getattr(_bu.compile_bir_kernel, "_is_cached_wrapper", False):
        return

    _orig = _bu.compile_bir_kernel

    _volatile = {"ant_traceback", "filename", "lineno", "kernel_name"}

    def _strip(obj):
        if isinstance(obj, dict):
            return {k: _strip(v) for k, v in obj.items() if k not in _volatile}
        if isinstance(obj, list):
            return [_strip(v) for v in obj]
        return obj

    def _key(bir_json):
        import orjson

        try:
            normalized = orjson.dumps(_strip(orjson.loads(bir_json)))
        except Exception:
            normalized = bir_json
        return hashlib.sha256(normalized).hexdigest()[:32]

    def _cached(bir_json, tmpdir, neff_name="file.neff"):
        import os as _os

        try:
            _os.makedirs(_NEFF_CACHE_DIR, exist_ok=True)
            p = _os.path.join(_NEFF_CACHE_DIR, _key(bir_json) + ".neff")
            dst = _os.path.join(tmpdir, neff_name)
            if _os.path.exists(p):
                shutil.copy(p, dst)
                return dst
            out = _orig(bir_json, tmpdir, neff_name)
            try:
                shutil.copy(out, p)
            except Exception:
                pass
            return out
        except Exception:
            return _orig(bir_json, tmpdir, neff_name)

    _cached._is_cached_wrapper = True
    _bu.compile_bir_kernel = _cached
    try:
        import concourse.bass2jax as _b2j

        if hasattr(_b2j, "compile_bir_kernel"):
            _b2j.compile_bir_kernel = _cached
    except Exception:
        pass


_install_neff_cache()

B, M, K = 1024, 64, 16
NCORES = 8
BL = B // NCORES
NPAIR = K // 2

_CACHE = {}
LAST_RESULTS = None
TRACE = False
PROBES = False  # embed one-shot timing probe instructions
VI_ENGINE = "vector"  # "vector" (broadcast-AP, 1x) | "gpsimd" (materialize on Pool)
REDUCE = "tree"  # "native" (tensor_reduce) | "tree" (pairwise adds)
# Tree levels with width <= TREE_POOL_W run on GpSimd (idle engine) instead
# of the saturated Vector engine. 0 disables the offload.
TREE_POOL_W = 8


def _build():
    nc = bacc.Bacc(
        "TRN2",
        target_bir_lowering=False,
        debug=False,
        num_devices=NCORES,
    )
    # x, k-major bf16: xb[b, k*M + i] = x[b, i, k]
    x_in = nc.declare_dram_parameter(
        "x", [BL, K * M], mybir.dt.bfloat16, isOutput=False
    )
    w_in = nc.declare_dram_parameter(
        "w", [1, M * M], mybir.dt.bfloat16, isOutput=False
    )
    # out, k-major fp32: out[b, k*M + i]
    out_ext = nc.declare_dram_parameter(
        "out", [BL, K * M], mybir.dt.float32, isOutput=True
    )

    with tile.TileContext(nc) as tc, ExitStack() as ctx:
        const = ctx.enter_context(tc.tile_pool(name="const", bufs=1))
        big = ctx.enter_context(tc.tile_pool(name="big", bufs=1))

        xb_sb = const.tile([BL, K * M], mybir.dt.bfloat16)
        nc.sync.dma_start(out=xb_sb[:, :], in_=x_in[:, :])

        w_sb = const.tile([BL, M * M], mybir.dt.bfloat16)
        w_bcast = bass.AP(
            tensor=w_in[0:1, :].tensor,
            offset=w_in[0:1, :].offset,
            ap=[[0, BL], [1, M * M]],
        )
        nc.gpsimd.dma_start(out=w_sb[:, :], in_=w_bcast)

        out_sb = const.tile([BL, K * M], mybir.dt.float32)

        xb_3d = xb_sb[:, :].rearrange("p (k i) -> p k i", i=M)
        w4 = (
            w_sb[:, :]
            .rearrange("p (i j) -> p i j", j=M)
            .unsqueeze(1)
            .broadcast_to((BL, 2, M, M))
        )

        # Pre-allocated ping-pong working tiles.
        vi_t = [
            big.tile([BL, 2, M, M], mybir.dt.bfloat16, tag=f"vi{p}", name=f"vi{p}")
            for p in range(2)
        ] if VI_ENGINE == "gpsimd" else None
        mt_t = [
            big.tile([BL, 2, M, M], mybir.dt.bfloat16, tag=f"mt{p}", name=f"mt{p}")
            for p in range(2)
        ]
        z_t = [
            big.tile([BL, 2, M, M], mybir.dt.bfloat16, tag=f"z{p}", name=f"z{p}")
            for p in range(2)
        ]
        e_t = [
            big.tile([BL, 2, M, M], mybir.dt.bfloat16, tag=f"e{p}", name=f"e{p}")
            for p in range(2)
        ]
        tr_t = None
        if REDUCE == "tree":
            tr_t = []
            for p in range(2):
                pair = []
                for s in range(2):
                    tiles = {}
                    width = M // 2
                    while width >= 2:
                        tiles[width] = big.tile(
                            [BL, 2 * M, width],
                            mybir.dt.bfloat16,
                            tag=f"tr{p}{s}{width}",
                            name=f"tr{p}{s}{width}",
                        )
                        width //= 2
                    pair.append(tiles)
                tr_t.append(pair)

        d_all = const.tile([BL, K * M], mybir.dt.float32, tag="dall", name="dall")
        n_all = const.tile([BL, K * M], mybir.dt.bfloat16, tag="nall", name="nall")
        rd_all = const.tile([BL, K * M], mybir.dt.float32, tag="rdall", name="rdall")
        p_all = const.tile([BL, K * M], mybir.dt.float32, tag="pall", name="pall")

        if PROBES:
            pr2 = const.tile([BL, 50], mybir.dt.float32, tag="pr2", name="pr2")

        for g in range(NPAIR):
            par = g % 2
            xp = xb_3d[:, 2 * g : 2 * g + 2, :]  # [BL, 2, M] contiguous
            vj = xp.unsqueeze(-2).broadcast_to((BL, 2, M, M))  # j innermost, s=1
            vi = xp.unsqueeze(-1).broadcast_to((BL, 2, M, M))  # j innermost, s=0

            m1 = mt_t[par]
            nc.vector.tensor_tensor(
                out=m1[:, :, :, :], in0=w4, in1=vj, op=mybir.AluOpType.mult
            )

            z = z_t[par]
            if VI_ENGINE == "gpsimd":
                vi_rep = vi_t[par]
                nc.gpsimd.tensor_scalar(
                    out=vi_rep[:, :, :, :],
                    in0=vi,
                    scalar1=1.0,
                    scalar2=None,
                    op0=mybir.AluOpType.mult,
                    op1=mybir.AluOpType.bypass,
                )
                vi_in = vi_rep[:, :, :, :]
            else:
                vi_in = vi  # direct broadcast AP (1x mode, proven pattern)
            nc.vector.tensor_tensor(
                out=z[:, :, :, :],
                in0=m1[:, :, :, :],
                in1=vi_in,
                op=mybir.AluOpType.mult,
            )

            e = e_t[par]
            nc.scalar.activation(
                out=e[:, :, :, :].rearrange("p a i j -> p (a i j)"),
                in_=z[:, :, :, :].rearrange("p a i j -> p (a i j)"),
                func=mybir.ActivationFunctionType.Exp,
            )

            # t overwrites m1 (m1 is dead after z).
            t = mt_t[par]
            nc.vector.tensor_tensor(
                out=t[:, :, :, :],
                in0=e[:, :, :, :],
                in1=vj,
                op=mybir.AluOpType.mult,
            )

            e_r = e[:, :, :, :].rearrange("p a i j -> p (a i) j")
            t_r = t[:, :, :, :].rearrange("p a i j -> p (a i) j")
            cols = slice(g * 2 * M, (g + 1) * 2 * M)
            if REDUCE == "native":
                with nc.allow_low_precision("bf16 softmax sums, 64-term rows"):
                    nc.vector.tensor_reduce(
                        out=d_all[:, cols],
                        in_=e_r,
                        axis=mybir.AxisListType.X,
                        op=mybir.AluOpType.add,
                    )
                    nc.vector.tensor_reduce(
                        out=n_all[:, cols],
                        in_=t_r,
                        axis=mybir.AxisListType.X,
                        op=mybir.AluOpType.add,
                    )
            else:
                for src, dst, tt in ((e_r, d_all, tr_t[par][0]), (t_r, n_all, tr_t[par][1])):
                    cur = src
                    width = M // 2
                    while width >= 2:
                        eng = nc.gpsimd if width <= TREE_POOL_W else nc.vector
                        nxt = tt[width][:, :, :]
                        eng.tensor_tensor(
                            out=nxt,
                            in0=cur[:, :, 0:width],
                            in1=cur[:, :, width : 2 * width],
                            op=mybir.AluOpType.add,
                        )
                        cur = nxt
                        width //= 2
                    eng = nc.gpsimd if 1 <= TREE_POOL_W else nc.vector
                    eng.tensor_tensor(
                        out=dst[:, cols],
                        in0=cur[:, :, 0],
                        in1=cur[:, :, 1],
                        op=mybir.AluOpType.add,
                    )

            if PROBES and g == 0:
                # One-shot instruction-cost probes (distinct free sizes;
                # standard ISA ops only). Pool input kept non-coalescable
                # (innermost count 63 < row stride 64) so the AP stays 3-dim
                # and the 5-dim pad in bass applies cleanly.
                w3 = w_sb[:, :].rearrange("p (a b) -> p a b", b=64)
                nc.vector.pool_avg(
                    out=pr2[:, :],
                    in_=w3[:, 0:50, 0:63],
                )

        nc.vector.reciprocal_approx_fast(out=rd_all[:, :], in_=d_all[:, :])
        nc.vector.tensor_tensor(
            out=p_all[:, :],
            in0=n_all[:, :],
            in1=rd_all[:, :],
            op=mybir.AluOpType.mult,
        )
        nc.vector.tensor_tensor(
            out=out_sb[:, :],
            in0=p_all[:, :],
            in1=xb_sb[:, :],
            op=mybir.AluOpType.mult,
        )

        nc.sync.dma_start(out=out_ext[:, :], in_=out_sb[:, :])

    nc.compile()
    return nc


def _get_nc():
    key = ("nc", PROBES, VI_ENGINE, REDUCE, TREE_POOL_W)
    if key not in _CACHE:
        _CACHE[key] = _build()
    return _CACHE[key]


def kernel(x, vk):
    global LAST_RESULTS
    x = np.ascontiguousarray(np.asarray(x), dtype=np.float32)
    vk = np.ascontiguousarray(np.asarray(vk), dtype=np.float32)
    assert x.shape == (B, M, K) and vk.shape[0] == M

    import ml_dtypes

    bf16 = ml_dtypes.bfloat16
    w = (vk @ vk.T).astype(bf16).reshape(1, M * M)
    # k-major bf16 per core: xb[b, k*M + i] = x[b, i, k]
    xs = x.reshape(NCORES, BL, M, K)
    in_maps = []
    for i in range(NCORES):
        xb = np.ascontiguousarray(xs[i].transpose(0, 2, 1)).reshape(BL, K * M)
        in_maps.append({"x": xb.astype(bf16), "w": w})

    nc = _get_nc()
    res = run_bass_kernel_spmd(nc, in_maps, core_ids=list(range(NCORES)), trace=TRACE)
    LAST_RESULTS = res
    outs = []
    for i in range(NCORES):
        o = np.asarray(res.results[i]["out"]).reshape(BL, K, M)
        outs.append(o.transpose(0, 2, 1))  # -> [BL, M, K]
    out = np.concatenate(outs, axis=0)
    return np.ascontiguousarray(out).astype(np.float32, copy=False)


# revision 29
# speedup vs baseline: 1.7089x; 1.3340x over previous
"""Trainium2 Bass kernel for nn_AttentionFFM — restructured v1.

Reference, per token (b, k) with v = x[b, :, k] (64-vector) and
constant symmetric w = vk @ vk.T:

    e_ij  = exp(v_i * v_j * w_ij)
    out_i = v_i * (sum_j v_j e_ij) / (sum_j e_ij)

(the v_i factor is pulled OUT of the softmax-weighted sum, so the
full-size q = s*e pass and the s = v_i*v_j outer product of the old
kernel disappear; only j-varying factors touch 64x64-sized tensors).

Layout (per core; batch-parallel across 8 cores, BL=128 batches each):
  partitions = batch b, free = (k2, i, j) for one k-PAIR at a time
  (k-pair packing halves instruction count). All big ops are bf16 with
  stride-1 innermost APs => DVE 2x mode:
    m1 = w  (*) v_j-broadcast        [DVE, fast AP]
    z  = m1 (*) vi_rep               [DVE, vi_rep materialized on GpSimd]
    e  = exp(z)                      [ACT]
    t  = e  (*) v_j-broadcast        [DVE, overwrites m1]
    D  = reduce_j(e), N = reduce_j(t)  [DVE native tensor_reduce, bf16 out]
  Final (batched over all 16 k): out = v * N * reciprocal(D) in fp32.

Host-side prep: x is passed k-major ((k,i) columns, bf16) so v-vectors
are contiguous per k; output is returned k-major and transposed back on
host. All tiles are allocated ONCE (tile-pool slot cycling faulted on
this HW in a previous session).
"""

import sys
from contextlib import ExitStack

import numpy as np

if "/opt/trn_rl_repo" not in sys.path:
    sys.path.insert(0, "/opt/trn_rl_repo")

import concourse.bass as bass
import concourse.tile as tile
from concourse import bacc, mybir
from concourse.bass_utils import run_bass_kernel_spmd

_NEFF_CACHE_DIR = "/tmp/bass_neff_cache"


def _install_neff_cache():
    import hashlib
    import shutil

    from concourse import bass_utils as _bu

    if getattr(_bu.compile_bir_kernel, "_is_cached_wrapper", False):
        return

    _orig = _bu.compile_bir_kernel

    _volatile = {"ant_traceback", "filename", "lineno", "kernel_name"}

    def _strip(obj):
        if isinstance(obj, dict):
            return {k: _strip(v) for k, v in obj.items() if k not in _volatile}
        if isinstance(obj, list):
            return [_strip(v) for v in obj]
        return obj

    def _key(bir_json):
        import orjson

        try:
            normalized = orjson.dumps(_strip(orjson.loads(bir_json)))
        except Exception:
            normalized = bir_json
        return hashlib.sha256(normalized).hexdigest()[:32]

    def _cached(bir_json, tmpdir, neff_name="file.neff"):
        import os as _os

        try:
            _os.makedirs(_NEFF_CACHE_DIR, exist_ok=True)
            p = _os.path.join(_NEFF_CACHE_DIR, _key(bir_json) + ".neff")
            dst = _os.path.join(tmpdir, neff_name)
            if _os.path.exists(p):
                shutil.copy(p, dst)
                return dst
            out = _orig(bir_json, tmpdir, neff_name)
            try:
                shutil.copy(out, p)
            except Exception:
                pass
            return out
        except Exception:
            return _orig(bir_json, tmpdir, neff_name)

    _cached._is_cached_wrapper = True
    _bu.compile_bir_kernel = _cached
    try:
        import concourse.bass2jax as _b2j

        if hasattr(_b2j, "compile_bir_kernel"):
            _b2j.compile_bir_kernel = _cached
    except Exception:
        pass


_install_neff_cache()

B, M, K = 1024, 64, 16
NCORES = 8
BL = B // NCORES
NPAIR = K // 2

_CACHE = {}
LAST_RESULTS = None
TRACE = False
PROBES = False  # embed one-shot timing probe instructions
# "vector": z reads the v_i broadcast AP directly (1x mode on DVE).
# "act": materialize vi_rep with an ACT-engine Copy (ACT cost is
#        stride-independent and ACT has idle capacity) => z runs 2x.
# "gpsimd": materialize on Pool (measured ~3-4x slower than cost model).
VI_ENGINE = "act"
# "native": vector.tensor_reduce (measured 1x — slow)
# "tree":   pairwise-halving bf16 adds (2x mode)
# "scan":   segmented prefix-sum via tensor_tensor_scan with a 0/1 reset
#           mask, row totals extracted from column j=63 by ACT copies
REDUCE = "tree"
# Tree levels with width <= TREE_POOL_W run on GpSimd. Measured: Pool
# tensor_tensor is ~4-7 ns/elem, useless => 0.
TREE_POOL_W = 0


def _build():
    nc = bacc.Bacc(
        "TRN2",
        target_bir_lowering=False,
        debug=False,
        num_devices=NCORES,
    )
    # x, k-major bf16: xb[b, k*M + i] = x[b, i, k]
    x_in = nc.declare_dram_parameter(
        "x", [BL, K * M], mybir.dt.bfloat16, isOutput=False
    )
    w_in = nc.declare_dram_parameter(
        "w", [1, M * M], mybir.dt.bfloat16, isOutput=False
    )
    m_in = None
    if REDUCE == "scan":
        # 0/1 reset mask, one 64-wide row (0 at j==0): broadcast along rows.
        m_in = nc.declare_dram_parameter(
            "m", [1, M], mybir.dt.bfloat16, isOutput=False
        )
    # out, k-major fp32: out[b, k*M + i]
    out_ext = nc.declare_dram_parameter(
        "out", [BL, K * M], mybir.dt.float32, isOutput=True
    )

    with tile.TileContext(nc) as tc, ExitStack() as ctx:
        const = ctx.enter_context(tc.tile_pool(name="const", bufs=1))
        big = ctx.enter_context(tc.tile_pool(name="big", bufs=1))

        xb_sb = const.tile([BL, K * M], mybir.dt.bfloat16)
        nc.sync.dma_start(out=xb_sb[:, :], in_=x_in[:, :])

        w_sb = const.tile([BL, M * M], mybir.dt.bfloat16)
        w_bcast = bass.AP(
            tensor=w_in[0:1, :].tensor,
            offset=w_in[0:1, :].offset,
            ap=[[0, BL], [1, M * M]],
        )
        nc.gpsimd.dma_start(out=w_sb[:, :], in_=w_bcast)

        out_sb = const.tile([BL, K * M], mybir.dt.float32)

        msk_view = None
        if REDUCE == "scan":
            msk_sb = const.tile([BL, M], mybir.dt.bfloat16)
            m_bcast = bass.AP(
                tensor=m_in[0:1, :].tensor,
                offset=m_in[0:1, :].offset,
                ap=[[0, BL], [1, M]],
            )
            nc.gpsimd.dma_start(out=msk_sb[:, :], in_=m_bcast)
            msk_view = msk_sb[:, :].unsqueeze(1).broadcast_to((BL, 2 * M, M))

        xb_3d = xb_sb[:, :].rearrange("p (k i) -> p k i", i=M)
        w4 = (
            w_sb[:, :]
            .rearrange("p (i j) -> p i j", j=M)
            .unsqueeze(1)
            .broadcast_to((BL, 2, M, M))
        )

        # Pre-allocated ping-pong working tiles.
        vi_t = [
            big.tile([BL, 2, M, M], mybir.dt.bfloat16, tag=f"vi{p}", name=f"vi{p}")
            for p in range(2)
        ] if VI_ENGINE in ("gpsimd", "act") else None
        mt_t = [
            big.tile([BL, 2, M, M], mybir.dt.bfloat16, tag=f"mt{p}", name=f"mt{p}")
            for p in range(2)
        ]
        z_t = [
            big.tile([BL, 2, M, M], mybir.dt.bfloat16, tag=f"z{p}", name=f"z{p}")
            for p in range(2)
        ]
        e_t = [
            big.tile([BL, 2, M, M], mybir.dt.bfloat16, tag=f"e{p}", name=f"e{p}")
            for p in range(2)
        ]
        tr_t = None
        if REDUCE == "tree":
            # One tree-tile set per parity, SHARED by the e- and t-trees
            # (they serialize on the Vector engine anyway; the Tile
            # dependency tracker orders the reuse).
            tr_t = []
            for p in range(2):
                tiles = {}
                width = M // 2
                while width >= 2:
                    tiles[width] = big.tile(
                        [BL, 2 * M, width],
                        mybir.dt.bfloat16,
                        tag=f"tr{p}{width}",
                        name=f"tr{p}{width}",
                    )
                    width //= 2
                tr_t.append((tiles, tiles))

        d_all = const.tile([BL, K * M], mybir.dt.float32, tag="dall", name="dall")
        n_all = const.tile([BL, K * M], mybir.dt.bfloat16, tag="nall", name="nall")
        rd_all = const.tile([BL, K * M], mybir.dt.float32, tag="rdall", name="rdall")
        p_all = const.tile([BL, K * M], mybir.dt.float32, tag="pall", name="pall")

        if PROBES:
            pr3 = const.tile([BL, 3584], mybir.dt.bfloat16, tag="pr3", name="pr3")
            pr5 = const.tile([BL, 2816], mybir.dt.bfloat16, tag="pr5", name="pr5")

        for g in range(NPAIR):
            par = g % 2
            xp = xb_3d[:, 2 * g : 2 * g + 2, :]  # [BL, 2, M] contiguous
            vj = xp.unsqueeze(-2).broadcast_to((BL, 2, M, M))  # j innermost, s=1
            vi = xp.unsqueeze(-1).broadcast_to((BL, 2, M, M))  # j innermost, s=0

            m1 = mt_t[par]
            nc.vector.tensor_tensor(
                out=m1[:, :, :, :], in0=w4, in1=vj, op=mybir.AluOpType.mult
            )

            z = z_t[par]
            if VI_ENGINE == "gpsimd":
                vi_rep = vi_t[par]
                nc.gpsimd.tensor_scalar(
                    out=vi_rep[:, :, :, :],
                    in0=vi,
                    scalar1=1.0,
                    scalar2=None,
                    op0=mybir.AluOpType.mult,
                    op1=mybir.AluOpType.bypass,
                )
                vi_in = vi_rep[:, :, :, :]
            elif VI_ENGINE == "act":
                vi_rep = vi_t[par]
                nc.scalar.copy(out=vi_rep[:, :, :, :], in_=vi)
                vi_in = vi_rep[:, :, :, :]
            else:
                vi_in = vi  # direct broadcast AP (1x mode, proven pattern)
            nc.vector.tensor_tensor(
                out=z[:, :, :, :],
                in0=m1[:, :, :, :],
                in1=vi_in,
                op=mybir.AluOpType.mult,
            )

            e = e_t[par]
            nc.scalar.activation(
                out=e[:, :, :, :].rearrange("p a i j -> p (a i j)"),
                in_=z[:, :, :, :].rearrange("p a i j -> p (a i j)"),
                func=mybir.ActivationFunctionType.Exp,
            )

            # t overwrites m1 (m1 is dead after z).
            t = mt_t[par]
            nc.vector.tensor_tensor(
                out=t[:, :, :, :],
                in0=e[:, :, :, :],
                in1=vj,
                op=mybir.AluOpType.mult,
            )

            e_r = e[:, :, :, :].rearrange("p a i j -> p (a i) j")
            t_r = t[:, :, :, :].rearrange("p a i j -> p (a i) j")
            cols = slice(g * 2 * M, (g + 1) * 2 * M)
            if REDUCE == "native":
                with nc.allow_low_precision("bf16 softmax sums, 64-term rows"):
                    nc.vector.tensor_reduce(
                        out=d_all[:, cols],
                        in_=e_r,
                        axis=mybir.AxisListType.X,
                        op=mybir.AluOpType.add,
                    )
                    nc.vector.tensor_reduce(
                        out=n_all[:, cols],
                        in_=t_r,
                        axis=mybir.AxisListType.X,
                        op=mybir.AluOpType.add,
                    )
            else:
                for src, dst, tt in ((e_r, d_all, tr_t[par][0]), (t_r, n_all, tr_t[par][1])):
                    cur = src
                    width = M // 2
                    while width >= 2:
                        eng = nc.gpsimd if width <= TREE_POOL_W else nc.vector
                        nxt = tt[width][:, :, :]
                        eng.tensor_tensor(
                            out=nxt,
                            in0=cur[:, :, 0:width],
                            in1=cur[:, :, width : 2 * width],
                            op=mybir.AluOpType.add,
                        )
                        cur = nxt
                        width //= 2
                    eng = nc.gpsimd if 1 <= TREE_POOL_W else nc.vector
                    eng.tensor_tensor(
                        out=dst[:, cols],
                        in0=cur[:, :, 0],
                        in1=cur[:, :, 1],
                        op=mybir.AluOpType.add,
                    )

            if PROBES and g == 0:
                # One-shot timing probes (distinct free sizes).
                w_flat = w_sb[:, :]
                nc.vector.tensor_tensor_scan(
                    out=pr3[:, :],
                    data0=w_flat[:, 0:3584],
                    data1=w_flat[:, 256:3840],
                    initial=0.0,
                    op0=mybir.AluOpType.mult,
                    op1=mybir.AluOpType.add,
                )
                nc.vector.scalar_tensor_tensor(
                    out=pr5[:, :],
                    in0=w_flat[:, 0:2816],
                    scalar=1.0,
                    in1=w_flat[:, 1024:3840],
                    op0=mybir.AluOpType.mult,
                    op1=mybir.AluOpType.mult,
                )

        nc.vector.reciprocal_approx_fast(out=rd_all[:, :], in_=d_all[:, :])
        nc.vector.tensor_tensor(
            out=p_all[:, :],
            in0=n_all[:, :],
            in1=rd_all[:, :],
            op=mybir.AluOpType.mult,
        )
        nc.vector.tensor_tensor(
            out=out_sb[:, :],
            in0=p_all[:, :],
            in1=xb_sb[:, :],
            op=mybir.AluOpType.mult,
        )

        nc.sync.dma_start(out=out_ext[:, :], in_=out_sb[:, :])

    nc.compile()
    return nc


def _get_nc():
    key = ("nc", PROBES, VI_ENGINE, REDUCE, TREE_POOL_W)
    if key not in _CACHE:
        _CACHE[key] = _build()
    return _CACHE[key]


def kernel(x, vk):
    global LAST_RESULTS
    x = np.ascontiguousarray(np.asarray(x), dtype=np.float32)
    vk = np.ascontiguousarray(np.asarray(vk), dtype=np.float32)
    assert x.shape == (B, M, K) and vk.shape[0] == M

    import ml_dtypes

    bf16 = ml_dtypes.bfloat16
    w = (vk @ vk.T).astype(bf16).reshape(1, M * M)
    # k-major bf16 per core: xb[b, k*M + i] = x[b, i, k]
    xs = x.reshape(NCORES, BL, M, K)
    in_maps = []
    for i in range(NCORES):
        xb = np.ascontiguousarray(xs[i].transpose(0, 2, 1)).reshape(BL, K * M)
        in_maps.append({"x": xb.astype(bf16), "w": w})

    nc = _get_nc()
    res = run_bass_kernel_spmd(nc, in_maps, core_ids=list(range(NCORES)), trace=TRACE)
    LAST_RESULTS = res
    outs = []
    for i in range(NCORES):
        o = np.asarray(res.results[i]["out"]).reshape(BL, K, M)
        outs.append(o.transpose(0, 2, 1))  # -> [BL, M, K]
    out = np.concatenate(outs, axis=0)
    return np.ascontiguousarray(out).astype(np.float32, copy=False)
